# revision 8
# baseline (speedup 1.0000x reference)
"""Trainium2 Bass kernel for a transformer encoder layer (B=2, S=2048, D=512,
H=8, D_FF=2048), returning (out, attention) like the reference.

Sharding: fully data-parallel over query tokens. 8 cores x 512 queries each
(core c -> batch c//4, query rows (c%4)*512 ...). Each core redundantly
computes K/V projections for its batch's full 2048-token sequence, so there
is no cross-core communication at all. Each core writes its slice of the
attention probabilities [8, 512, 2048] and the output [512, 512].

All compute is fp32 (trn2's PE streams fp32 at the same per-column rate as
bf16, so there is no precision/performance tradeoff worth taking here).
"""

import sys

for _p in ("/opt/trn_rl_repo",):
    if _p not in sys.path:
        sys.path.insert(0, _p)

from contextlib import ExitStack

import numpy as np

import concourse.bass as bass
import concourse.bacc as bacc
import concourse.mybir as mybir
import concourse.tile as tile
from concourse.bass_utils import run_bass_kernel_spmd
from concourse.masks import make_identity

FP = mybir.dt.float32
AF = mybir.ActivationFunctionType
ALU = mybir.AluOpType
AX = mybir.AxisListType

P = 128
D, S, H, DEP, F = 512, 2048, 8, 64, 2048
NQ = 512  # queries per core
DC, FC, SC, QS = D // P, F // P, S // P, NQ // P  # 4, 16, 16, 4
N_CORES = 8
EPS = 1e-5

_NC_CACHE = {}


def _layernorm(nc, work, src, out_ap, g_b, be_b, eps_t):
    """LayerNorm over the free dim (D=512) of a [128, D] tile."""
    s = work.tile([P, 1], FP, tag="ln_s")
    nc.vector.tensor_reduce(s[:], src[:], axis=AX.X, op=ALU.add)
    negmean = work.tile([P, 1], FP, tag="ln_nm")
    nc.vector.tensor_scalar_mul(negmean[:], s[:], -1.0 / D)
    cen = work.tile([P, D], FP, tag="ln_cen")
    nc.vector.tensor_scalar_add(cen[:], src[:], negmean[:])
    sq = work.tile([P, D], FP, tag="ln_sq")
    vsum = work.tile([P, 1], FP, tag="ln_vs")
    nc.scalar.activation(sq[:], cen[:], AF.Square, accum_out=vsum[:])
    std = work.tile([P, 1], FP, tag="ln_std")
    # std = sqrt(vsum / D + eps)
    nc.scalar.activation(std[:], vsum[:], AF.Sqrt, scale=1.0 / D, bias=eps_t[:])
    rstd = work.tile([P, 1], FP, tag="ln_rstd")
    nc.vector.reciprocal(rstd[:], std[:])
    nc.vector.tensor_scalar_mul(cen[:], cen[:], rstd[:])
    nc.vector.tensor_tensor(cen[:], cen[:], g_b[:], ALU.mult)
    nc.vector.tensor_tensor(out_ap, cen[:], be_b[:], ALU.add)


def _body(ctx, tc, t):
    nc = tc.nc

    consts = ctx.enter_context(tc.tile_pool(name="consts", bufs=1))
    persist = ctx.enter_context(tc.tile_pool(name="persist", bufs=1))
    work = ctx.enter_context(tc.tile_pool(name="work", bufs=3))

    ident = consts.tile([P, P], FP)
    make_identity(nc, ident)
    ones_row = consts.tile([1, DEP], FP)
    nc.vector.memset(ones_row, 1.0)
    eps_t = consts.tile([P, 1], FP)
    nc.vector.memset(eps_t, EPS)

    def row_bcast(ap_1d, n):
        # [P, n] tile: the 1-D dram vector replicated across all partitions
        tl = consts.tile([P, n], FP, tag=f"rb_{ap_1d.tensor.name}")
        src = bass.AP(
            tensor=ap_1d.tensor,
            offset=ap_1d.offset,
            ap=[[0, P]] + [list(x) for x in ap_1d.ap],
        )
        nc.gpsimd.dma_start(out=tl[:], in_=src)
        return tl

    bv_b = row_bcast(t["bv"], D)
    bo_b = row_bcast(t["bo"], D)
    b2_b = row_bcast(t["b2"], D)
    g1_b = row_bcast(t["ln1_g"], D)
    be1_b = row_bcast(t["ln1_b"], D)
    g2_b = row_bcast(t["ln2_g"], D)
    be2_b = row_bcast(t["ln2_b"], D)

    def col_tile(ap_1d, nchunks):
        # [P, nchunks] tile: element (p, c) = vec[c*128 + p]
        tl = consts.tile([P, nchunks], FP, tag=f"ct_{ap_1d.tensor.name}")
        nc.gpsimd.dma_start(out=tl[:], in_=ap_1d.rearrange("(o p) -> p o", p=P))
        return tl

    bq_c = col_tile(t["bq"], DC)
    bk_c = col_tile(t["bk"], DC)
    b1_c = col_tile(t["b1"], FC)

    # persistent activations
    xq_s = persist.tile([P, QS, D], FP)  # residual stream slice, token-major
    nc.sync.dma_start(xq_s[:], t["xq"].rearrange("(o p) f -> p o f", p=P))
    kt_s = persist.tile([P, DC, S], FP)  # K^T: [hd % 128, hd // 128, key]
    v_s = persist.tile([P, SC, D], FP)  # V: [key % 128, key // 128, hd]
    h1_s = persist.tile([P, QS, D], FP)  # post-LN1 activations, token-major
    h1T_s = persist.tile([P, DC, NQ], FP)  # same, transposed

    # tiles needed through phase 3 only (released before the FFN phase)
    mid = ctx.enter_context(ExitStack())
    midp = mid.enter_context(tc.tile_pool(name="midp", bufs=1))
    wo_s = midp.tile([P, DC, D], FP)
    nc.sync.dma_start(wo_s[:], t["wo"].rearrange("(o p) f -> p o f", p=P))
    qt_s = midp.tile([P, DC, NQ], FP)  # Q^T: [hd % 128, hd // 128, q]
    ctxT_s = midp.tile([P, DC, NQ], FP)  # context^T: [hd % 128, hd // 128, q]

    # ---- phase 1: Q/K/V projections -------------------------------------
    with (
        tc.tile_pool(name="qkv_sb", bufs=1) as qkv_sb,
        tc.tile_pool(name="wstream1", bufs=3) as wstream,
        tc.tile_pool(name="ps_proj", bufs=4, space="PSUM") as psp,
    ):
        xfT_s = qkv_sb.tile([P, DC, S], FP)
        nc.sync.dma_start(xfT_s[:], t["xfT"].rearrange("(o p) f -> p o f", p=P))
        xqT_s = qkv_sb.tile([P, DC, NQ], FP)
        nc.sync.dma_start(xqT_s[:], t["xqT"].rearrange("(o p) f -> p o f", p=P))
        wv_s = qkv_sb.tile([P, DC, D], FP)
        nc.sync.dma_start(wv_s[:], t["wv"].rearrange("(o p) f -> p o f", p=P))

        # Q^T[c*128:(c+1)*128, :] = sum_dc wq[dc, c].T @ xq^T[dc, :]
        for c in range(DC):
            pt = psp.tile([P, NQ], FP, tag="pj")
            for dc in range(DC):
                wt = wstream.tile([P, P], FP, tag="wchunk")
                nc.sync.dma_start(wt[:], t["wq"][dc * P : (dc + 1) * P, c * P : (c + 1) * P])
                nc.tensor.matmul(pt[:], lhsT=wt[:], rhs=xqT_s[:, dc, :], start=(dc == 0), stop=(dc == DC - 1))
            nc.scalar.activation(qt_s[:, c, :], pt[:], AF.Identity, bias=bq_c[:, c : c + 1])

        # K^T over the full sequence
        for c in range(DC):
            for s4 in range(S // 512):
                pt = psp.tile([P, 512], FP, tag="pj")
                for dc in range(DC):
                    wt = wstream.tile([P, P], FP, tag="wchunk")
                    nc.sync.dma_start(wt[:], t["wk"][dc * P : (dc + 1) * P, c * P : (c + 1) * P])
                    nc.tensor.matmul(
                        pt[:],
                        lhsT=wt[:],
                        rhs=xfT_s[:, dc, s4 * 512 : (s4 + 1) * 512],
                        start=(dc == 0),
                        stop=(dc == DC - 1),
                    )
                nc.scalar.activation(
                    kt_s[:, c, s4 * 512 : (s4 + 1) * 512], pt[:], AF.Identity, bias=bk_c[:, c : c + 1]
                )

        # V token-major: V[tok chunk] = sum_dc xfT[:, dc, chunk].T @ wv[dc, :]
        for k16 in range(SC):
            pt = psp.tile([P, D], FP, tag="pj")
            for dc in range(DC):
                nc.tensor.matmul(
                    pt[:],
                    lhsT=xfT_s[:, dc, k16 * P : (k16 + 1) * P],
                    rhs=wv_s[:, dc, :],
                    start=(dc == 0),
                    stop=(dc == DC - 1),
                )
            nc.vector.tensor_tensor(v_s[:, k16, :], pt[:], bv_b[:], ALU.add)

    # ---- phase 2: attention ---------------------------------------------
    with (
        tc.tile_pool(name="awork", bufs=3) as awork,
        tc.tile_pool(name="bigwork", bufs=2) as bigwork,
        tc.tile_pool(name="pa", bufs=1, space="PSUM") as pa,
        tc.tile_pool(name="pb", bufs=2, space="PSUM") as pb,
        tc.tile_pool(name="pc", bufs=1, space="PSUM") as pc,
        tc.tile_pool(name="pr", bufs=1, space="PSUM") as pr,
    ):
        for h in range(H):
            hc, hr = h // 2, (h % 2) * DEP
            qt_h = qt_s[hr : hr + DEP, hc, :]  # [64, NQ]
            kt_h = kt_s[hr : hr + DEP, hc, :]  # [64, S]
            invrow = awork.tile([1, NQ], FP, tag="invrow", name="invrow")

            # path A: scores in [q, key] layout -> softmax -> HBM
            for q4 in range(QS):
                ps = pa.tile([P, S], FP, tag="pa")
                for s4 in range(S // 512):
                    nc.tensor.matmul(
                        ps[:, s4 * 512 : (s4 + 1) * 512],
                        lhsT=qt_h[:, q4 * P : (q4 + 1) * P],
                        rhs=kt_h[:, s4 * 512 : (s4 + 1) * 512],
                        start=True,
                        stop=True,
                    )
                expq = bigwork.tile([P, S], FP, tag="expq")
                sums4 = awork.tile([P, S // 512], FP, tag="sums4", name="sums4")
                for s4 in range(S // 512):
                    nc.scalar.activation(
                        expq[:, s4 * 512 : (s4 + 1) * 512],
                        ps[:, s4 * 512 : (s4 + 1) * 512],
                        AF.Exp,
                        scale=0.125,
                        accum_out=sums4[:, s4 : s4 + 1],
                    )
                sums = awork.tile([P, 1], FP, tag="sums", name="sums")
                nc.vector.tensor_reduce(sums[:], sums4[:], axis=AX.X, op=ALU.add)
                inv = awork.tile([P, 1], FP, tag="inv", name="inv")
                nc.vector.reciprocal(inv[:], sums[:])
                nc.vector.tensor_scalar_mul(expq[:], expq[:], inv[:])
                nc.sync.dma_start(t["attn_o"][h, q4 * P : (q4 + 1) * P, :], expq[:])
                # transpose inv [128,1] -> row [1,128] via matmul with identity
                prt = pr.tile([1, P], FP, tag="pr")
                nc.tensor.matmul(prt[:], lhsT=inv[:], rhs=ident[:], start=True, stop=True)
                nc.scalar.activation(invrow[0:1, q4 * P : (q4 + 1) * P], prt[:], AF.Copy)

            # broadcast inv row across 64 partitions (for scaling context^T)
            pib = pb.tile([DEP, NQ], FP, tag="pb")
            nc.tensor.matmul(pib[:], lhsT=ones_row[:], rhs=invrow[0:1, :], start=True, stop=True)
            invb = awork.tile([DEP, NQ], FP, tag="invb", name="invb")
            nc.scalar.activation(invb[:], pib[:], AF.Copy)

            # path B: scores^T in [key, q] layout -> exp -> context^T
            pctx = pc.tile([DEP, NQ], FP, tag="pc")
            for k16 in range(SC):
                pst = pb.tile([P, NQ], FP, tag="pb")
                nc.tensor.matmul(pst[:], lhsT=kt_h[:, k16 * P : (k16 + 1) * P], rhs=qt_h[:], start=True, stop=True)
                expt = awork.tile([P, NQ], FP, tag="expt", name="expt")
                nc.scalar.activation(expt[:], pst[:], AF.Exp, scale=0.125)
                nc.tensor.matmul(
                    pctx[:],
                    lhsT=v_s[:, k16, h * DEP : (h + 1) * DEP],
                    rhs=expt[:],
                    start=(k16 == 0),
                    stop=(k16 == SC - 1),
                )
            nc.vector.tensor_tensor(ctxT_s[hr : hr + DEP, hc, :], pctx[:], invb[:], ALU.mult)

    # ---- phase 3: output projection + LN1 + transpose -------------------
    with (
        tc.tile_pool(name="ps_d", bufs=2, space="PSUM") as psd,
        tc.tile_pool(name="ps_t", bufs=2, space="PSUM") as pstp,
    ):
        for q4 in range(QS):
            pao = psd.tile([P, D], FP, tag="pao")
            for c in range(DC):
                nc.tensor.matmul(
                    pao[:],
                    lhsT=ctxT_s[:, c, q4 * P : (q4 + 1) * P],
                    rhs=wo_s[:, c, :],
                    start=(c == 0),
                    stop=(c == DC - 1),
                )
            tmp = work.tile([P, D], FP, tag="tok_tmp")
            nc.vector.tensor_tensor(tmp[:], pao[:], bo_b[:], ALU.add)
            nc.vector.tensor_tensor(tmp[:], tmp[:], xq_s[:, q4, :], ALU.add)
            _layernorm(nc, work, tmp, h1_s[:, q4, :], g1_b, be1_b, eps_t)
            for c in range(DC):
                ptt = pstp.tile([P, P], FP, tag="ptt")
                nc.tensor.transpose(ptt[:], h1_s[:, q4, c * P : (c + 1) * P], ident[:])
                nc.scalar.activation(h1T_s[:, c, q4 * P : (q4 + 1) * P], ptt[:], AF.Copy)

    mid.close()  # free wo/qt/ctxT SBUF for the FFN phase

    # ---- phase 4: FFN + LN2 ---------------------------------------------
    with (
        tc.tile_pool(name="wstream2", bufs=3) as wstream,
        tc.tile_pool(name="ps_f", bufs=2, space="PSUM") as psf,
        tc.tile_pool(name="ps_o", bufs=1, space="PSUM") as pso,
    ):
        pouts = [pso.tile([P, D], FP, tag=f"po{q4}", name=f"po{q4}") for q4 in range(QS)]
        for fc in range(FC):
            pf = psf.tile([P, NQ], FP, tag="pf")
            for dc in range(DC):
                w1t = wstream.tile([P, P], FP, tag="w1c")
                nc.sync.dma_start(w1t[:], t["w1"][dc * P : (dc + 1) * P, fc * P : (fc + 1) * P])
                nc.tensor.matmul(pf[:], lhsT=w1t[:], rhs=h1T_s[:, dc, :], start=(dc == 0), stop=(dc == DC - 1))
            ft = work.tile([P, NQ], FP, tag="ft")
            nc.scalar.activation(ft[:], pf[:], AF.Relu, bias=b1_c[:, fc : fc + 1])
            w2t = wstream.tile([P, D], FP, tag="w2c")
            nc.sync.dma_start(w2t[:], t["w2"][fc * P : (fc + 1) * P, :])
            for q4 in range(QS):
                nc.tensor.matmul(
                    pouts[q4][:],
                    lhsT=ft[:, q4 * P : (q4 + 1) * P],
                    rhs=w2t[:],
                    start=(fc == 0),
                    stop=(fc == FC - 1),
                )
        for q4 in range(QS):
            tmp = work.tile([P, D], FP, tag="tok_tmp")
            nc.vector.tensor_tensor(tmp[:], pouts[q4][:], b2_b[:], ALU.add)
            nc.vector.tensor_tensor(tmp[:], tmp[:], h1_s[:, q4, :], ALU.add)
            outt = work.tile([P, D], FP, tag="outt")
            _layernorm(nc, work, tmp, outt[:], g2_b, be2_b, eps_t)
            nc.sync.dma_start(t["out_o"][q4 * P : (q4 + 1) * P, :], outt[:])


def build_nc():
    if "nc" in _NC_CACHE:
        return _NC_CACHE["nc"]
    nc = bacc.Bacc("TRN2", target_bir_lowering=False, debug=False)
    t = {}

    def din(name, shape):
        t[name] = nc.dram_tensor(name, shape, FP, kind="ExternalInput").ap()

    din("xfT", [D, S])
    din("xqT", [D, NQ])
    din("xq", [NQ, D])
    for w in ("wq", "wk", "wv", "wo"):
        din(w, [D, D])
    for b in ("bq", "bk", "bv", "bo", "b2", "ln1_g", "ln1_b", "ln2_g", "ln2_b"):
        din(b, [D])
    din("w1", [D, F])
    din("b1", [F])
    din("w2", [F, D])
    t["attn_o"] = nc.dram_tensor("attn_o", [H, NQ, S], FP, kind="ExternalOutput").ap()
    t["out_o"] = nc.dram_tensor("out_o", [NQ, D], FP, kind="ExternalOutput").ap()

    with tile.TileContext(nc) as tc:
        with ExitStack() as ctx:
            _body(ctx, tc, t)
    nc.compile()
    _NC_CACHE["nc"] = nc
    return nc


def make_in_maps(inputs):
    """Shard the full inputs into 8 per-core input maps."""
    x = np.asarray(inputs["x"], dtype=np.float32)
    shared = {}
    for k in ("wq", "wk", "wv", "wo", "bq", "bk", "bv", "bo", "b2", "ln1_g", "ln1_b", "ln2_g", "ln2_b", "w1", "b1", "w2"):
        shared[k] = np.ascontiguousarray(np.asarray(inputs[k], dtype=np.float32))
    in_maps = []
    for c in range(N_CORES):
        b, q0 = c // 4, (c % 4) * NQ
        m = dict(shared)
        m["xfT"] = np.ascontiguousarray(x[b].T)
        m["xqT"] = np.ascontiguousarray(x[b, q0 : q0 + NQ, :].T)
        m["xq"] = np.ascontiguousarray(x[b, q0 : q0 + NQ, :])
        in_maps.append(m)
    return in_maps


def assemble(results):
    out = np.empty((2, S, D), dtype=np.float32)
    attention = np.empty((2, H, S, S), dtype=np.float32)
    for c in range(N_CORES):
        b, q0 = c // 4, (c % 4) * NQ
        out[b, q0 : q0 + NQ, :] = results[c]["out_o"]
        attention[b, :, q0 : q0 + NQ, :] = results[c]["attn_o"]
    return out, attention


def kernel(**inputs):
    nc = build_nc()
    in_maps = make_in_maps(inputs)
    res = run_bass_kernel_spmd(nc, in_maps, list(range(N_CORES)))
    return assemble(res.results)


if __name__ == "__main__":
    rng = np.random.default_rng(0)
    ins = {
        "x": rng.standard_normal((2, S, D), dtype=np.float32),
        "wq": rng.standard_normal((D, D), dtype=np.float32) * 0.02,
        "bq": np.zeros(D, np.float32),
        "wk": rng.standard_normal((D, D), dtype=np.float32) * 0.02,
        "bk": np.zeros(D, np.float32),
        "wv": rng.standard_normal((D, D), dtype=np.float32) * 0.02,
        "bv": np.zeros(D, np.float32),
        "wo": rng.standard_normal((D, D), dtype=np.float32) * 0.02,
        "bo": np.zeros(D, np.float32),
        "w1": rng.standard_normal((D, F), dtype=np.float32) * 0.02,
        "b1": np.zeros(F, np.float32),
        "w2": rng.standard_normal((F, D), dtype=np.float32) * 0.02,
        "b2": np.zeros(D, np.float32),
        "ln1_g": np.ones(D, np.float32),
        "ln1_b": np.zeros(D, np.float32),
        "ln2_g": np.ones(D, np.float32),
        "ln2_b": np.zeros(D, np.float32),
    }
    out, attention = kernel(**ins)
    print("out", out.shape, out.dtype, "attention", attention.shape, attention.dtype)


# revision 9
# speedup vs baseline: 1.6156x; 1.6156x over previous
"""Trainium2 Bass kernel for a transformer encoder layer (B=2, S=2048, D=512,
H=8, D_FF=2048), returning (out, attention) like the reference.

Sharding: fully data-parallel over query tokens. 8 cores x 512 queries each
(core c -> batch c//4, query rows (c%4)*512 ...). Each core redundantly
computes K/V projections for its batch's full 2048-token sequence, so there
is no cross-core communication at all. Each core writes its slice of the
attention probabilities [8, 512, 2048] and the output [512, 512].

All compute is fp32 (trn2's PE streams fp32 at the same per-column rate as
bf16, so there is no precision/performance tradeoff worth taking here).
"""

import sys

for _p in ("/opt/trn_rl_repo",):
    if _p not in sys.path:
        sys.path.insert(0, _p)

from contextlib import ExitStack

import numpy as np

import concourse.bass as bass
import concourse.bacc as bacc
import concourse.mybir as mybir
import concourse.tile as tile
from concourse.bass_utils import run_bass_kernel_spmd
from concourse.masks import make_identity

FP = mybir.dt.float32
FPR = mybir.dt.float32r  # tf32-like: 1 cycle/row on PE (vs 4 for fp32) at N>=256
AF = mybir.ActivationFunctionType
ALU = mybir.AluOpType
AX = mybir.AxisListType

P = 128
D, S, H, DEP, F = 512, 2048, 8, 64, 2048
NQ = 512  # queries per core
DC, FC, SC, QS = D // P, F // P, S // P, NQ // P  # 4, 16, 16, 4
N_CORES = 8
EPS = 1e-5

_NC_CACHE = {}


def _layernorm(nc, work, src, out_ap, g_b, be_b, eps_t):
    """LayerNorm over the free dim (D=512) of a [128, D] tile."""
    s = work.tile([P, 1], FP, tag="ln_s")
    nc.vector.tensor_reduce(s[:], src[:], axis=AX.X, op=ALU.add)
    negmean = work.tile([P, 1], FP, tag="ln_nm")
    nc.vector.tensor_scalar_mul(negmean[:], s[:], -1.0 / D)
    cen = work.tile([P, D], FP, tag="ln_cen")
    nc.vector.tensor_scalar_add(cen[:], src[:], negmean[:])
    sq = work.tile([P, D], FP, tag="ln_sq")
    vsum = work.tile([P, 1], FP, tag="ln_vs")
    nc.scalar.activation(sq[:], cen[:], AF.Square, accum_out=vsum[:])
    std = work.tile([P, 1], FP, tag="ln_std")
    # std = sqrt(vsum / D + eps)
    nc.scalar.activation(std[:], vsum[:], AF.Sqrt, scale=1.0 / D, bias=eps_t[:])
    rstd = work.tile([P, 1], FP, tag="ln_rstd")
    nc.vector.reciprocal(rstd[:], std[:])
    nc.vector.tensor_scalar_mul(cen[:], cen[:], rstd[:])
    nc.vector.tensor_tensor(cen[:], cen[:], g_b[:], ALU.mult)
    nc.vector.tensor_tensor(out_ap, cen[:], be_b[:], ALU.add)


def _body(ctx, tc, t):
    nc = tc.nc

    consts = ctx.enter_context(tc.tile_pool(name="consts", bufs=1))
    persist = ctx.enter_context(tc.tile_pool(name="persist", bufs=1))
    work = ctx.enter_context(tc.tile_pool(name="work", bufs=3))

    ident = consts.tile([P, P], FP)
    make_identity(nc, ident)
    ones_f32 = consts.tile([1, DEP], FP)
    nc.vector.memset(ones_f32, 1.0)
    ones_row = consts.tile([1, DEP], FPR)
    nc.scalar.activation(ones_row[:], ones_f32[:], AF.Copy)
    eps_t = consts.tile([P, 1], FP)
    nc.vector.memset(eps_t, EPS)

    def row_bcast(ap_1d, n):
        # [P, n] tile: the 1-D dram vector replicated across all partitions
        tl = consts.tile([P, n], FP, tag=f"rb_{ap_1d.tensor.name}")
        src = bass.AP(
            tensor=ap_1d.tensor,
            offset=ap_1d.offset,
            ap=[[0, P]] + [list(x) for x in ap_1d.ap],
        )
        nc.gpsimd.dma_start(out=tl[:], in_=src)
        return tl

    bv_b = row_bcast(t["bv"], D)
    bo_b = row_bcast(t["bo"], D)
    b2_b = row_bcast(t["b2"], D)
    g1_b = row_bcast(t["ln1_g"], D)
    be1_b = row_bcast(t["ln1_b"], D)
    g2_b = row_bcast(t["ln2_g"], D)
    be2_b = row_bcast(t["ln2_b"], D)

    def col_tile(ap_1d, nchunks):
        # [P, nchunks] tile: element (p, c) = vec[c*128 + p]
        tl = consts.tile([P, nchunks], FP, tag=f"ct_{ap_1d.tensor.name}")
        nc.gpsimd.dma_start(out=tl[:], in_=ap_1d.rearrange("(o p) -> p o", p=P))
        return tl

    bq_c = col_tile(t["bq"], DC)
    bk_c = col_tile(t["bk"], DC)
    b1_c = col_tile(t["b1"], FC)

    # persistent activations
    xq_s = persist.tile([P, QS, D], FP)  # residual stream slice, token-major
    nc.sync.dma_start(xq_s[:], t["xq"].rearrange("(o p) f -> p o f", p=P))
    kt_s = persist.tile([P, DC, S], FPR)  # K^T: [hd % 128, hd // 128, key]
    v_s = persist.tile([P, SC, D], FPR)  # V: [key % 128, key // 128, hd]
    h1_s = persist.tile([P, QS, D], FP)  # post-LN1 activations, token-major
    h1T_s = persist.tile([P, DC, NQ], FPR)  # same, transposed

    # tiles needed through phase 3 only (released before the FFN phase)
    mid = ctx.enter_context(ExitStack())
    midp = mid.enter_context(tc.tile_pool(name="midp", bufs=1))
    wo_s = midp.tile([P, DC, D], FPR)
    nc.sync.dma_start(wo_s[:], t["wo"].rearrange("(o p) f -> p o f", p=P))
    qt_s = midp.tile([P, DC, NQ], FPR)  # Q^T: [hd % 128, hd // 128, q]
    ctxT_s = midp.tile([P, DC, NQ], FPR)  # context^T: [hd % 128, hd // 128, q]

    # ---- phase 1: Q/K/V projections -------------------------------------
    with (
        tc.tile_pool(name="qkv_sb", bufs=1) as qkv_sb,
        tc.tile_pool(name="wstream1", bufs=3) as wstream,
        tc.tile_pool(name="ps_proj", bufs=4, space="PSUM") as psp,
    ):
        xfT_s = qkv_sb.tile([P, DC, S], FPR)
        nc.sync.dma_start(xfT_s[:], t["xfT"].rearrange("(o p) f -> p o f", p=P))
        xqT_s = qkv_sb.tile([P, DC, NQ], FPR)
        nc.sync.dma_start(xqT_s[:], t["xqT"].rearrange("(o p) f -> p o f", p=P))
        wv_s = qkv_sb.tile([P, DC, D], FPR)
        nc.sync.dma_start(wv_s[:], t["wv"].rearrange("(o p) f -> p o f", p=P))

        # Q^T[c*128:(c+1)*128, :] = sum_dc wq[dc, c].T @ xq^T[dc, :]
        for c in range(DC):
            pt = psp.tile([P, NQ], FP, tag="pj")
            for dc in range(DC):
                wt = wstream.tile([P, P], FPR, tag="wchunk")
                nc.sync.dma_start(wt[:], t["wq"][dc * P : (dc + 1) * P, c * P : (c + 1) * P])
                nc.tensor.matmul(pt[:], lhsT=wt[:], rhs=xqT_s[:, dc, :], start=(dc == 0), stop=(dc == DC - 1))
            nc.scalar.activation(qt_s[:, c, :], pt[:], AF.Identity, bias=bq_c[:, c : c + 1])

        # K^T over the full sequence
        for c in range(DC):
            for s4 in range(S // 512):
                pt = psp.tile([P, 512], FP, tag="pj")
                for dc in range(DC):
                    wt = wstream.tile([P, P], FPR, tag="wchunk")
                    nc.sync.dma_start(wt[:], t["wk"][dc * P : (dc + 1) * P, c * P : (c + 1) * P])
                    nc.tensor.matmul(
                        pt[:],
                        lhsT=wt[:],
                        rhs=xfT_s[:, dc, s4 * 512 : (s4 + 1) * 512],
                        start=(dc == 0),
                        stop=(dc == DC - 1),
                    )
                nc.scalar.activation(
                    kt_s[:, c, s4 * 512 : (s4 + 1) * 512], pt[:], AF.Identity, bias=bk_c[:, c : c + 1]
                )

        # V token-major: V[tok chunk] = sum_dc xfT[:, dc, chunk].T @ wv[dc, :]
        for k16 in range(SC):
            pt = psp.tile([P, D], FP, tag="pj")
            for dc in range(DC):
                nc.tensor.matmul(
                    pt[:],
                    lhsT=xfT_s[:, dc, k16 * P : (k16 + 1) * P],
                    rhs=wv_s[:, dc, :],
                    start=(dc == 0),
                    stop=(dc == DC - 1),
                )
            nc.vector.tensor_tensor(v_s[:, k16, :], pt[:], bv_b[:], ALU.add)

    # ---- phase 2: attention ---------------------------------------------
    with (
        tc.tile_pool(name="awork", bufs=3) as awork,
        tc.tile_pool(name="bigwork", bufs=2) as bigwork,
        tc.tile_pool(name="pa", bufs=1, space="PSUM") as pa,
        tc.tile_pool(name="pb", bufs=2, space="PSUM") as pb,
        tc.tile_pool(name="pc", bufs=1, space="PSUM") as pc,
        tc.tile_pool(name="pr", bufs=1, space="PSUM") as pr,
    ):
        for h in range(H):
            hc, hr = h // 2, (h % 2) * DEP
            qt_h = qt_s[hr : hr + DEP, hc, :]  # [64, NQ]
            kt_h = kt_s[hr : hr + DEP, hc, :]  # [64, S]
            invrow = awork.tile([1, NQ], FPR, tag="invrow", name="invrow")

            # path A: scores in [q, key] layout -> softmax -> HBM
            for q4 in range(QS):
                ps = pa.tile([P, S], FP, tag="pa")
                for s4 in range(S // 512):
                    nc.tensor.matmul(
                        ps[:, s4 * 512 : (s4 + 1) * 512],
                        lhsT=qt_h[:, q4 * P : (q4 + 1) * P],
                        rhs=kt_h[:, s4 * 512 : (s4 + 1) * 512],
                        start=True,
                        stop=True,
                    )
                expq = bigwork.tile([P, S], FP, tag="expq")
                sums4 = awork.tile([P, S // 512], FP, tag="sums4", name="sums4")
                for s4 in range(S // 512):
                    nc.scalar.activation(
                        expq[:, s4 * 512 : (s4 + 1) * 512],
                        ps[:, s4 * 512 : (s4 + 1) * 512],
                        AF.Exp,
                        scale=0.125,
                        accum_out=sums4[:, s4 : s4 + 1],
                    )
                sums = awork.tile([P, 1], FP, tag="sums", name="sums")
                nc.vector.tensor_reduce(sums[:], sums4[:], axis=AX.X, op=ALU.add)
                inv = awork.tile([P, 1], FP, tag="inv", name="inv")
                nc.vector.reciprocal(inv[:], sums[:])
                nc.vector.tensor_scalar_mul(expq[:], expq[:], inv[:])
                nc.sync.dma_start(t["attn_o"][h, q4 * P : (q4 + 1) * P, :], expq[:])
                # transpose inv [128,1] -> row [1,128] via matmul with identity
                prt = pr.tile([1, P], FP, tag="pr")
                nc.tensor.matmul(prt[:], lhsT=inv[:], rhs=ident[:], start=True, stop=True)
                nc.scalar.activation(invrow[0:1, q4 * P : (q4 + 1) * P], prt[:], AF.Copy)

            # broadcast inv row across 64 partitions (for scaling context^T)
            pib = pb.tile([DEP, NQ], FP, tag="pb")
            nc.tensor.matmul(pib[:], lhsT=ones_row[:], rhs=invrow[0:1, :], start=True, stop=True)
            invb = awork.tile([DEP, NQ], FP, tag="invb", name="invb")
            nc.scalar.activation(invb[:], pib[:], AF.Copy)

            # path B: scores^T in [key, q] layout -> exp -> context^T
            pctx = pc.tile([DEP, NQ], FP, tag="pc")
            for k16 in range(SC):
                pst = pb.tile([P, NQ], FP, tag="pb")
                nc.tensor.matmul(pst[:], lhsT=kt_h[:, k16 * P : (k16 + 1) * P], rhs=qt_h[:], start=True, stop=True)
                expt = awork.tile([P, NQ], FPR, tag="expt", name="expt")
                nc.scalar.activation(expt[:], pst[:], AF.Exp, scale=0.125)
                nc.tensor.matmul(
                    pctx[:],
                    lhsT=v_s[:, k16, h * DEP : (h + 1) * DEP],
                    rhs=expt[:],
                    start=(k16 == 0),
                    stop=(k16 == SC - 1),
                )
            nc.vector.tensor_tensor(ctxT_s[hr : hr + DEP, hc, :], pctx[:], invb[:], ALU.mult)

    # ---- phase 3: output projection + LN1 + transpose -------------------
    with (
        tc.tile_pool(name="ps_d", bufs=2, space="PSUM") as psd,
        tc.tile_pool(name="ps_t", bufs=2, space="PSUM") as pstp,
    ):
        for q4 in range(QS):
            pao = psd.tile([P, D], FP, tag="pao")
            for c in range(DC):
                nc.tensor.matmul(
                    pao[:],
                    lhsT=ctxT_s[:, c, q4 * P : (q4 + 1) * P],
                    rhs=wo_s[:, c, :],
                    start=(c == 0),
                    stop=(c == DC - 1),
                )
            tmp = work.tile([P, D], FP, tag="tok_tmp")
            nc.vector.tensor_tensor(tmp[:], pao[:], bo_b[:], ALU.add)
            nc.vector.tensor_tensor(tmp[:], tmp[:], xq_s[:, q4, :], ALU.add)
            _layernorm(nc, work, tmp, h1_s[:, q4, :], g1_b, be1_b, eps_t)
            for c in range(DC):
                ptt = pstp.tile([P, P], FP, tag="ptt")
                nc.tensor.transpose(ptt[:], h1_s[:, q4, c * P : (c + 1) * P], ident[:])
                nc.scalar.activation(h1T_s[:, c, q4 * P : (q4 + 1) * P], ptt[:], AF.Copy)

    mid.close()  # free wo/qt/ctxT SBUF for the FFN phase

    # ---- phase 4: FFN + LN2 ---------------------------------------------
    with (
        tc.tile_pool(name="wstream2", bufs=3) as wstream,
        tc.tile_pool(name="ps_f", bufs=2, space="PSUM") as psf,
        tc.tile_pool(name="ps_o", bufs=1, space="PSUM") as pso,
    ):
        pouts = [pso.tile([P, D], FP, tag=f"po{q4}", name=f"po{q4}") for q4 in range(QS)]
        for fc in range(FC):
            pf = psf.tile([P, NQ], FP, tag="pf")
            for dc in range(DC):
                w1t = wstream.tile([P, P], FPR, tag="w1c")
                nc.sync.dma_start(w1t[:], t["w1"][dc * P : (dc + 1) * P, fc * P : (fc + 1) * P])
                nc.tensor.matmul(pf[:], lhsT=w1t[:], rhs=h1T_s[:, dc, :], start=(dc == 0), stop=(dc == DC - 1))
            ft = work.tile([P, NQ], FPR, tag="ft")
            nc.scalar.activation(ft[:], pf[:], AF.Relu, bias=b1_c[:, fc : fc + 1])
            w2t = wstream.tile([P, D], FPR, tag="w2c")
            nc.sync.dma_start(w2t[:], t["w2"][fc * P : (fc + 1) * P, :])
            for q4 in range(QS):
                nc.tensor.matmul(
                    pouts[q4][:],
                    lhsT=ft[:, q4 * P : (q4 + 1) * P],
                    rhs=w2t[:],
                    start=(fc == 0),
                    stop=(fc == FC - 1),
                )
        for q4 in range(QS):
            tmp = work.tile([P, D], FP, tag="tok_tmp")
            nc.vector.tensor_tensor(tmp[:], pouts[q4][:], b2_b[:], ALU.add)
            nc.vector.tensor_tensor(tmp[:], tmp[:], h1_s[:, q4, :], ALU.add)
            outt = work.tile([P, D], FP, tag="outt")
            _layernorm(nc, work, tmp, outt[:], g2_b, be2_b, eps_t)
            nc.sync.dma_start(t["out_o"][q4 * P : (q4 + 1) * P, :], outt[:])


def build_nc():
    if "nc" in _NC_CACHE:
        return _NC_CACHE["nc"]
    nc = bacc.Bacc("TRN2", target_bir_lowering=False, debug=False)
    t = {}

    def din(name, shape, dt=FP):
        t[name] = nc.dram_tensor(name, shape, dt, kind="ExternalInput").ap()

    din("xfT", [D, S], FPR)
    din("xqT", [D, NQ], FPR)
    din("xq", [NQ, D])
    for w in ("wq", "wk", "wv", "wo"):
        din(w, [D, D], FPR)
    for b in ("bq", "bk", "bv", "bo", "b2", "ln1_g", "ln1_b", "ln2_g", "ln2_b"):
        din(b, [D])
    din("w1", [D, F], FPR)
    din("b1", [F])
    din("w2", [F, D], FPR)
    t["attn_o"] = nc.dram_tensor("attn_o", [H, NQ, S], FP, kind="ExternalOutput").ap()
    t["out_o"] = nc.dram_tensor("out_o", [NQ, D], FP, kind="ExternalOutput").ap()

    with tile.TileContext(nc) as tc:
        with ExitStack() as ctx:
            _body(ctx, tc, t)
    nc.compile()
    _NC_CACHE["nc"] = nc
    return nc


def make_in_maps(inputs):
    """Shard the full inputs into 8 per-core input maps."""
    x = np.asarray(inputs["x"], dtype=np.float32)
    shared = {}
    for k in ("wq", "wk", "wv", "wo", "bq", "bk", "bv", "bo", "b2", "ln1_g", "ln1_b", "ln2_g", "ln2_b", "w1", "b1", "w2"):
        shared[k] = np.ascontiguousarray(np.asarray(inputs[k], dtype=np.float32))
    in_maps = []
    for c in range(N_CORES):
        b, q0 = c // 4, (c % 4) * NQ
        m = dict(shared)
        m["xfT"] = np.ascontiguousarray(x[b].T)
        m["xqT"] = np.ascontiguousarray(x[b, q0 : q0 + NQ, :].T)
        m["xq"] = np.ascontiguousarray(x[b, q0 : q0 + NQ, :])
        in_maps.append(m)
    return in_maps


def assemble(results):
    out = np.empty((2, S, D), dtype=np.float32)
    attention = np.empty((2, H, S, S), dtype=np.float32)
    for c in range(N_CORES):
        b, q0 = c // 4, (c % 4) * NQ
        out[b, q0 : q0 + NQ, :] = results[c]["out_o"]
        attention[b, :, q0 : q0 + NQ, :] = results[c]["attn_o"]
    return out, attention


def kernel(**inputs):
    nc = build_nc()
    in_maps = make_in_maps(inputs)
    res = run_bass_kernel_spmd(nc, in_maps, list(range(N_CORES)))
    return assemble(res.results)


if __name__ == "__main__":
    rng = np.random.default_rng(0)
    ins = {
        "x": rng.standard_normal((2, S, D), dtype=np.float32),
        "wq": rng.standard_normal((D, D), dtype=np.float32) * 0.02,
        "bq": np.zeros(D, np.float32),
        "wk": rng.standard_normal((D, D), dtype=np.float32) * 0.02,
        "bk": np.zeros(D, np.float32),
        "wv": rng.standard_normal((D, D), dtype=np.float32) * 0.02,
        "bv": np.zeros(D, np.float32),
        "wo": rng.standard_normal((D, D), dtype=np.float32) * 0.02,
        "bo": np.zeros(D, np.float32),
        "w1": rng.standard_normal((D, F), dtype=np.float32) * 0.02,
        "b1": np.zeros(F, np.float32),
        "w2": rng.standard_normal((F, D), dtype=np.float32) * 0.02,
        "b2": np.zeros(D, np.float32),
        "ln1_g": np.ones(D, np.float32),
        "ln1_b": np.zeros(D, np.float32),
        "ln2_g": np.ones(D, np.float32),
        "ln2_b": np.zeros(D, np.float32),
    }
    out, attention = kernel(**ins)
    print("out", out.shape, out.dtype, "attention", attention.shape, attention.dtype)


# revision 10
# speedup vs baseline: 1.8907x; 1.1703x over previous
"""Trainium2 Bass kernel for a transformer encoder layer (B=2, S=2048, D=512,
H=8, D_FF=2048), returning (out, attention) like the reference.

Sharding: fully data-parallel over query tokens. 8 cores x 512 queries each
(core c -> batch c//4, query rows (c%4)*512 ...). Each core redundantly
computes K/V projections for its batch's full 2048-token sequence, so there
is no cross-core communication at all. Each core writes its slice of the
attention probabilities [8, 512, 2048] and the output [512, 512].

All compute is fp32 (trn2's PE streams fp32 at the same per-column rate as
bf16, so there is no precision/performance tradeoff worth taking here).
"""

import sys

for _p in ("/opt/trn_rl_repo",):
    if _p not in sys.path:
        sys.path.insert(0, _p)

from contextlib import ExitStack

import numpy as np

import concourse.bass as bass
import concourse.bacc as bacc
import concourse.mybir as mybir
import concourse.tile as tile
from concourse.bass_utils import run_bass_kernel_spmd
from concourse.masks import make_identity

FP = mybir.dt.float32
FPR = mybir.dt.float32r  # tf32-like: 1 cycle/row on PE (vs 4 for fp32) at N>=256
AF = mybir.ActivationFunctionType
ALU = mybir.AluOpType
AX = mybir.AxisListType

P = 128
D, S, H, DEP, F = 512, 2048, 8, 64, 2048
NQ = 512  # queries per core
DC, FC, SC, QS = D // P, F // P, S // P, NQ // P  # 4, 16, 16, 4
N_CORES = 8
EPS = 1e-5

_NC_CACHE = {}


def _layernorm(nc, work, src, out_ap, g_b, be_b, eps_t):
    """LayerNorm over the free dim (D=512) of a [128, D] tile."""
    s = work.tile([P, 1], FP, tag="ln_s")
    nc.vector.tensor_reduce(s[:], src[:], axis=AX.X, op=ALU.add)
    negmean = work.tile([P, 1], FP, tag="ln_nm")
    nc.vector.tensor_scalar_mul(negmean[:], s[:], -1.0 / D)
    cen = work.tile([P, D], FP, tag="ln_cen")
    nc.vector.tensor_scalar_add(cen[:], src[:], negmean[:])
    sq = work.tile([P, D], FP, tag="ln_sq")
    vsum = work.tile([P, 1], FP, tag="ln_vs")
    nc.scalar.activation(sq[:], cen[:], AF.Square, accum_out=vsum[:])
    std = work.tile([P, 1], FP, tag="ln_std")
    # std = sqrt(vsum / D + eps)
    nc.scalar.activation(std[:], vsum[:], AF.Sqrt, scale=1.0 / D, bias=eps_t[:])
    rstd = work.tile([P, 1], FP, tag="ln_rstd")
    nc.vector.reciprocal(rstd[:], std[:])
    nc.vector.tensor_scalar_mul(cen[:], cen[:], rstd[:])
    nc.vector.tensor_tensor(cen[:], cen[:], g_b[:], ALU.mult)
    nc.vector.tensor_tensor(out_ap, cen[:], be_b[:], ALU.add)


def _body(ctx, tc, t):
    nc = tc.nc

    consts = ctx.enter_context(tc.tile_pool(name="consts", bufs=1))
    persist = ctx.enter_context(tc.tile_pool(name="persist", bufs=1))
    work = ctx.enter_context(tc.tile_pool(name="work", bufs=3))

    ident = consts.tile([P, P], FP)
    make_identity(nc, ident)
    ones_f32 = consts.tile([1, DEP], FP)
    nc.vector.memset(ones_f32, 1.0)
    ones_row = consts.tile([1, DEP], FPR)
    nc.scalar.activation(ones_row[:], ones_f32[:], AF.Copy)
    eps_t = consts.tile([P, 1], FP)
    nc.vector.memset(eps_t, EPS)

    def row_bcast(ap_1d, n):
        # [P, n] tile: the 1-D dram vector replicated across all partitions
        tl = consts.tile([P, n], FP, tag=f"rb_{ap_1d.tensor.name}")
        src = bass.AP(
            tensor=ap_1d.tensor,
            offset=ap_1d.offset,
            ap=[[0, P]] + [list(x) for x in ap_1d.ap],
        )
        nc.gpsimd.dma_start(out=tl[:], in_=src)
        return tl

    bv_b = row_bcast(t["bv"], D)
    bo_b = row_bcast(t["bo"], D)
    b2_b = row_bcast(t["b2"], D)
    g1_b = row_bcast(t["ln1_g"], D)
    be1_b = row_bcast(t["ln1_b"], D)
    g2_b = row_bcast(t["ln2_g"], D)
    be2_b = row_bcast(t["ln2_b"], D)

    def col_tile(ap_1d, nchunks):
        # [P, nchunks] tile: element (p, c) = vec[c*128 + p]
        tl = consts.tile([P, nchunks], FP, tag=f"ct_{ap_1d.tensor.name}")
        nc.gpsimd.dma_start(out=tl[:], in_=ap_1d.rearrange("(o p) -> p o", p=P))
        return tl

    bq_c = col_tile(t["bq"], DC)
    bk_c = col_tile(t["bk"], DC)
    b1_c = col_tile(t["b1"], FC)

    # persistent activations
    xq_s = persist.tile([P, QS, D], FP)  # residual stream slice, token-major
    nc.sync.dma_start(xq_s[:], t["xq"].rearrange("(o p) f -> p o f", p=P))
    kt_s = persist.tile([P, DC, S], FPR)  # K^T: [hd % 128, hd // 128, key]
    v_s = persist.tile([P, SC, D], FPR)  # V: [key % 128, key // 128, hd]
    h1_s = persist.tile([P, QS, D], FP)  # post-LN1 activations, token-major
    h1T_s = persist.tile([P, DC, NQ], FPR)  # same, transposed

    # tiles needed through phase 3 only (released before the FFN phase)
    mid = ctx.enter_context(ExitStack())
    midp = mid.enter_context(tc.tile_pool(name="midp", bufs=1))
    wo_s = midp.tile([P, DC, D], FPR)
    nc.sync.dma_start(wo_s[:], t["wo"].rearrange("(o p) f -> p o f", p=P))
    qt_s = midp.tile([P, DC, NQ], FPR)  # Q^T: [hd % 128, hd // 128, q]
    ctxT_s = midp.tile([P, DC, NQ], FPR)  # context^T: [hd % 128, hd // 128, q]

    # ---- phase 1: Q/K/V projections -------------------------------------
    with (
        tc.tile_pool(name="qkv_sb", bufs=1) as qkv_sb,
        tc.tile_pool(name="wstream1", bufs=3) as wstream,
        tc.tile_pool(name="ps_proj", bufs=4, space="PSUM") as psp,
    ):
        xfT_s = qkv_sb.tile([P, DC, S], FPR)
        nc.sync.dma_start(xfT_s[:], t["xfT"].rearrange("(o p) f -> p o f", p=P))
        xqT_s = qkv_sb.tile([P, DC, NQ], FPR)
        nc.sync.dma_start(xqT_s[:], t["xqT"].rearrange("(o p) f -> p o f", p=P))
        wv_s = qkv_sb.tile([P, DC, D], FPR)
        nc.sync.dma_start(wv_s[:], t["wv"].rearrange("(o p) f -> p o f", p=P))

        # Q^T[c*128:(c+1)*128, :] = sum_dc wq[dc, c].T @ xq^T[dc, :]
        for c in range(DC):
            pt = psp.tile([P, NQ], FP, tag="pj")
            for dc in range(DC):
                wt = wstream.tile([P, P], FPR, tag="wchunk")
                nc.sync.dma_start(wt[:], t["wq"][dc * P : (dc + 1) * P, c * P : (c + 1) * P])
                nc.tensor.matmul(pt[:], lhsT=wt[:], rhs=xqT_s[:, dc, :], start=(dc == 0), stop=(dc == DC - 1))
            nc.scalar.activation(qt_s[:, c, :], pt[:], AF.Identity, bias=bq_c[:, c : c + 1])

        # K^T over the full sequence
        for c in range(DC):
            for s4 in range(S // 512):
                pt = psp.tile([P, 512], FP, tag="pj")
                for dc in range(DC):
                    wt = wstream.tile([P, P], FPR, tag="wchunk")
                    nc.sync.dma_start(wt[:], t["wk"][dc * P : (dc + 1) * P, c * P : (c + 1) * P])
                    nc.tensor.matmul(
                        pt[:],
                        lhsT=wt[:],
                        rhs=xfT_s[:, dc, s4 * 512 : (s4 + 1) * 512],
                        start=(dc == 0),
                        stop=(dc == DC - 1),
                    )
                nc.scalar.activation(
                    kt_s[:, c, s4 * 512 : (s4 + 1) * 512], pt[:], AF.Identity, bias=bk_c[:, c : c + 1]
                )

        # V token-major: V[tok chunk] = sum_dc xfT[:, dc, chunk].T @ wv[dc, :]
        for k16 in range(SC):
            pt = psp.tile([P, D], FP, tag="pj")
            for dc in range(DC):
                nc.tensor.matmul(
                    pt[:],
                    lhsT=xfT_s[:, dc, k16 * P : (k16 + 1) * P],
                    rhs=wv_s[:, dc, :],
                    start=(dc == 0),
                    stop=(dc == DC - 1),
                )
            nc.vector.tensor_tensor(v_s[:, k16, :], pt[:], bv_b[:], ALU.add)

    # ---- phase 2: attention ---------------------------------------------
    with (
        tc.tile_pool(name="awork", bufs=3) as awork,
        tc.tile_pool(name="bigwork", bufs=2) as bigwork,
        tc.tile_pool(name="pa", bufs=4, space="PSUM") as pa,
        tc.tile_pool(name="pb", bufs=2, space="PSUM") as pb,
        tc.tile_pool(name="pc", bufs=2, space="PSUM") as pc,
    ):
        for h in range(H):
            hc, hr = h // 2, (h % 2) * DEP
            qt_h = qt_s[hr : hr + DEP, hc, :]  # [64, NQ]
            kt_h = kt_s[hr : hr + DEP, hc, :]  # [64, S]
            invrow = awork.tile([1, NQ], FPR, tag="invrow", name="invrow")
            pctx = pc.tile([DEP, NQ], FP, tag="pc", name="pctx")

            # Path A (scores -> softmax -> HBM) and path B (scores^T -> exp
            # -> context^T) interleaved so the PE queue stays dense: path-B
            # matmuls fill the gaps while path-A PSUM banks drain through
            # the Scalar engine's exp.
            for q4 in range(QS):
                expq = bigwork.tile([P, S], FP, tag="expq", name="expq")
                sums4 = awork.tile([P, S // 512], FP, tag="sums4", name="sums4")
                for s4 in range(S // 512):
                    ps = pa.tile([P, 512], FP, tag="pa", name="ps")
                    nc.tensor.matmul(
                        ps[:],
                        lhsT=qt_h[:, q4 * P : (q4 + 1) * P],
                        rhs=kt_h[:, s4 * 512 : (s4 + 1) * 512],
                        start=True,
                        stop=True,
                    )
                    nc.scalar.activation(
                        expq[:, s4 * 512 : (s4 + 1) * 512],
                        ps[:],
                        AF.Exp,
                        scale=0.125,
                        accum_out=sums4[:, s4 : s4 + 1],
                    )
                sums = awork.tile([P, 1], FP, tag="sums", name="sums")
                nc.vector.tensor_reduce(sums[:], sums4[:], axis=AX.X, op=ALU.add)
                inv = awork.tile([P, 1], FP, tag="inv", name="inv")
                nc.vector.reciprocal(inv[:], sums[:])
                nc.vector.tensor_scalar_mul(expq[:], expq[:], inv[:])
                nc.sync.dma_start(t["attn_o"][h, q4 * P : (q4 + 1) * P, :], expq[:])
                # transpose inv [128,1] -> row [1,128] via matmul with identity
                prt = pb.tile([1, P], FP, tag="pb", name="prt")
                nc.tensor.matmul(prt[:], lhsT=inv[:], rhs=ident[:], start=True, stop=True)
                nc.scalar.activation(invrow[0:1, q4 * P : (q4 + 1) * P], prt[:], AF.Copy)

                # path B: 4 of the 16 key chunks per q4 round
                for k16 in range(q4 * 4, q4 * 4 + 4):
                    pst = pb.tile([P, NQ], FP, tag="pb", name="pst")
                    nc.tensor.matmul(pst[:], lhsT=kt_h[:, k16 * P : (k16 + 1) * P], rhs=qt_h[:], start=True, stop=True)
                    expt = awork.tile([P, NQ], FPR, tag="expt", name="expt")
                    nc.scalar.activation(expt[:], pst[:], AF.Exp, scale=0.125)
                    nc.tensor.matmul(
                        pctx[:],
                        lhsT=v_s[:, k16, h * DEP : (h + 1) * DEP],
                        rhs=expt[:],
                        start=(k16 == 0),
                        stop=(k16 == SC - 1),
                    )

            # broadcast inv row across 64 partitions (for scaling context^T)
            pib = pb.tile([DEP, NQ], FP, tag="pb", name="pib")
            nc.tensor.matmul(pib[:], lhsT=ones_row[:], rhs=invrow[0:1, :], start=True, stop=True)
            invb = awork.tile([DEP, NQ], FP, tag="invb", name="invb")
            nc.scalar.activation(invb[:], pib[:], AF.Copy)
            nc.vector.tensor_tensor(ctxT_s[hr : hr + DEP, hc, :], pctx[:], invb[:], ALU.mult)

    # ---- phase 3: output projection + LN1 + transpose -------------------
    with (
        tc.tile_pool(name="ps_d", bufs=2, space="PSUM") as psd,
        tc.tile_pool(name="ps_t", bufs=2, space="PSUM") as pstp,
    ):
        for q4 in range(QS):
            pao = psd.tile([P, D], FP, tag="pao")
            for c in range(DC):
                nc.tensor.matmul(
                    pao[:],
                    lhsT=ctxT_s[:, c, q4 * P : (q4 + 1) * P],
                    rhs=wo_s[:, c, :],
                    start=(c == 0),
                    stop=(c == DC - 1),
                )
            tmp = work.tile([P, D], FP, tag="tok_tmp")
            nc.vector.tensor_tensor(tmp[:], pao[:], bo_b[:], ALU.add)
            nc.vector.tensor_tensor(tmp[:], tmp[:], xq_s[:, q4, :], ALU.add)
            _layernorm(nc, work, tmp, h1_s[:, q4, :], g1_b, be1_b, eps_t)
            for c in range(DC):
                ptt = pstp.tile([P, P], FP, tag="ptt")
                nc.tensor.transpose(ptt[:], h1_s[:, q4, c * P : (c + 1) * P], ident[:])
                nc.scalar.activation(h1T_s[:, c, q4 * P : (q4 + 1) * P], ptt[:], AF.Copy)

    mid.close()  # free wo/qt/ctxT SBUF for the FFN phase

    # ---- phase 4: FFN + LN2 ---------------------------------------------
    with (
        tc.tile_pool(name="wstream2", bufs=3) as wstream,
        tc.tile_pool(name="ps_f", bufs=2, space="PSUM") as psf,
        tc.tile_pool(name="ps_o", bufs=1, space="PSUM") as pso,
    ):
        pouts = [pso.tile([P, D], FP, tag=f"po{q4}", name=f"po{q4}") for q4 in range(QS)]
        for fc in range(FC):
            pf = psf.tile([P, NQ], FP, tag="pf")
            for dc in range(DC):
                w1t = wstream.tile([P, P], FPR, tag="w1c")
                nc.sync.dma_start(w1t[:], t["w1"][dc * P : (dc + 1) * P, fc * P : (fc + 1) * P])
                nc.tensor.matmul(pf[:], lhsT=w1t[:], rhs=h1T_s[:, dc, :], start=(dc == 0), stop=(dc == DC - 1))
            ft = work.tile([P, NQ], FPR, tag="ft")
            nc.scalar.activation(ft[:], pf[:], AF.Relu, bias=b1_c[:, fc : fc + 1])
            w2t = wstream.tile([P, D], FPR, tag="w2c")
            nc.sync.dma_start(w2t[:], t["w2"][fc * P : (fc + 1) * P, :])
            for q4 in range(QS):
                nc.tensor.matmul(
                    pouts[q4][:],
                    lhsT=ft[:, q4 * P : (q4 + 1) * P],
                    rhs=w2t[:],
                    start=(fc == 0),
                    stop=(fc == FC - 1),
                )
        for q4 in range(QS):
            tmp = work.tile([P, D], FP, tag="tok_tmp")
            nc.vector.tensor_tensor(tmp[:], pouts[q4][:], b2_b[:], ALU.add)
            nc.vector.tensor_tensor(tmp[:], tmp[:], h1_s[:, q4, :], ALU.add)
            outt = work.tile([P, D], FP, tag="outt")
            _layernorm(nc, work, tmp, outt[:], g2_b, be2_b, eps_t)
            nc.sync.dma_start(t["out_o"][q4 * P : (q4 + 1) * P, :], outt[:])


def build_nc():
    if "nc" in _NC_CACHE:
        return _NC_CACHE["nc"]
    nc = bacc.Bacc("TRN2", target_bir_lowering=False, debug=False)
    t = {}

    def din(name, shape, dt=FP):
        t[name] = nc.dram_tensor(name, shape, dt, kind="ExternalInput").ap()

    din("xfT", [D, S], FPR)
    din("xqT", [D, NQ], FPR)
    din("xq", [NQ, D])
    for w in ("wq", "wk", "wv", "wo"):
        din(w, [D, D], FPR)
    for b in ("bq", "bk", "bv", "bo", "b2", "ln1_g", "ln1_b", "ln2_g", "ln2_b"):
        din(b, [D])
    din("w1", [D, F], FPR)
    din("b1", [F])
    din("w2", [F, D], FPR)
    t["attn_o"] = nc.dram_tensor("attn_o", [H, NQ, S], FP, kind="ExternalOutput").ap()
    t["out_o"] = nc.dram_tensor("out_o", [NQ, D], FP, kind="ExternalOutput").ap()

    with tile.TileContext(nc) as tc:
        with ExitStack() as ctx:
            _body(ctx, tc, t)
    nc.compile()
    _NC_CACHE["nc"] = nc
    return nc


def make_in_maps(inputs):
    """Shard the full inputs into 8 per-core input maps."""
    x = np.asarray(inputs["x"], dtype=np.float32)
    shared = {}
    for k in ("wq", "wk", "wv", "wo", "bq", "bk", "bv", "bo", "b2", "ln1_g", "ln1_b", "ln2_g", "ln2_b", "w1", "b1", "w2"):
        shared[k] = np.ascontiguousarray(np.asarray(inputs[k], dtype=np.float32))
    in_maps = []
    for c in range(N_CORES):
        b, q0 = c // 4, (c % 4) * NQ
        m = dict(shared)
        m["xfT"] = np.ascontiguousarray(x[b].T)
        m["xqT"] = np.ascontiguousarray(x[b, q0 : q0 + NQ, :].T)
        m["xq"] = np.ascontiguousarray(x[b, q0 : q0 + NQ, :])
        in_maps.append(m)
    return in_maps


def assemble(results):
    out = np.empty((2, S, D), dtype=np.float32)
    attention = np.empty((2, H, S, S), dtype=np.float32)
    for c in range(N_CORES):
        b, q0 = c // 4, (c % 4) * NQ
        out[b, q0 : q0 + NQ, :] = results[c]["out_o"]
        attention[b, :, q0 : q0 + NQ, :] = results[c]["attn_o"]
    return out, attention


def kernel(**inputs):
    nc = build_nc()
    in_maps = make_in_maps(inputs)
    res = run_bass_kernel_spmd(nc, in_maps, list(range(N_CORES)))
    return assemble(res.results)


if __name__ == "__main__":
    rng = np.random.default_rng(0)
    ins = {
        "x": rng.standard_normal((2, S, D), dtype=np.float32),
        "wq": rng.standard_normal((D, D), dtype=np.float32) * 0.02,
        "bq": np.zeros(D, np.float32),
        "wk": rng.standard_normal((D, D), dtype=np.float32) * 0.02,
        "bk": np.zeros(D, np.float32),
        "wv": rng.standard_normal((D, D), dtype=np.float32) * 0.02,
        "bv": np.zeros(D, np.float32),
        "wo": rng.standard_normal((D, D), dtype=np.float32) * 0.02,
        "bo": np.zeros(D, np.float32),
        "w1": rng.standard_normal((D, F), dtype=np.float32) * 0.02,
        "b1": np.zeros(F, np.float32),
        "w2": rng.standard_normal((F, D), dtype=np.float32) * 0.02,
        "b2": np.zeros(D, np.float32),
        "ln1_g": np.ones(D, np.float32),
        "ln1_b": np.zeros(D, np.float32),
        "ln2_g": np.ones(D, np.float32),
        "ln2_b": np.zeros(D, np.float32),
    }
    out, attention = kernel(**ins)
    print("out", out.shape, out.dtype, "attention", attention.shape, attention.dtype)
